# revision 8
# baseline (speedup 1.0000x reference)
"""Trainium2 Bass kernel for nn_CayleyConv.

Reference computation (per batch b):
  G = cayley(g[:24])                                   # [9,3,3]
  Y[c,h,w,k] = G[k] @ X[c,h,w] @ G[k]^T                # 3x3 mats per pixel
  O[c,h2,w2] = sum_k Y[c,h2-a,w2-b,k]  (k=(a,b))       # 3x3 overlap-add fold
  out[h2,w2,:,:,o] = sum_c O[c,h2,w2] * c2[c,o]        # channel mix

Strategy (data-parallel over batch, 1 batch per NeuronCore):
  * transform+fold in ONE PE pass: stationary W_k[(c',jl),(im,c')] =
    delta_{c'} * M_k[im,jl] (M_k = kron(G_k,G_k)) gives K=126 contraction
    (14 channels x 9 matrix components on partitions), fold realized by
    9 shift-accumulated matmuls into one PSUM tile per 15-row output half.
  * input zero-padded to 32x32 spatially so every fold matmul writes the
    identical full PSUM region (uniform has_written semantics).
  * partition reorg (im,c') -> c via on-chip DMA into o2 [c, im, pix2].
  * channel mix: lhsT = o2 2D-chunk (pix2-major, im-minor = final row
    order), rhs = c2 [c,256]; psum-accumulate the two 128-channel halves.
  * float32r matmuls (full-rate fp32 streaming for N>=256).
"""

import numpy as np

import concourse.bass as bass
import concourse.mybir as mybir
import concourse.tile as tile
from concourse import bacc
from concourse.bass_utils import run_bass_kernel_spmd

# ---------------- problem constants (hardcoded per contract) ----------------
B = 8
C_IN = 256
C_OUT = 256
H = 28
W = 28
NCB = 19          # ceil(256/14) channel blocks
CPB = 14          # channels per block
PTF = 126         # CPB * 9 partitions for transform+fold
PIX2 = 900        # 30*30 output pixels
NROW = PIX2 * 9   # 8100 output rows (pix2-major, im-minor)
EPS = 1e-7

USE_F32R = True   # float32r matmuls (1 cyc/row at N>=256) vs float32 (4 cyc/row)


# ---------------- host-side param prep ----------------
def _inv3(m):
    b00, b01, b02 = m[..., 0, 0], m[..., 0, 1], m[..., 0, 2]
    b10, b11, b12 = m[..., 1, 0], m[..., 1, 1], m[..., 1, 2]
    b20, b21, b22 = m[..., 2, 0], m[..., 2, 1], m[..., 2, 2]
    det = (b00 * (b11 * b22 - b12 * b21)
           - b01 * (b10 * b22 - b12 * b20)
           + b02 * (b10 * b21 - b11 * b20))
    adj = np.stack([
        np.stack([b11 * b22 - b12 * b21, b02 * b21 - b01 * b22, b01 * b12 - b02 * b11], axis=-1),
        np.stack([b12 * b20 - b10 * b22, b00 * b22 - b02 * b20, b02 * b10 - b00 * b12], axis=-1),
        np.stack([b10 * b21 - b11 * b20, b01 * b20 - b00 * b21, b00 * b11 - b01 * b10], axis=-1),
    ], axis=-2)
    return adj / (det + EPS)[..., None, None]


def _cayley(k24):
    p = k24.reshape(8, 3)
    a, b, c = p[:, 0], p[:, 1], p[:, 2]
    one = np.ones_like(a)
    first = np.stack([
        np.stack([one, -a, -b], axis=-1),
        np.stack([a, one, -c], axis=-1),
        np.stack([b, c, one], axis=-1),
    ], axis=-2)
    second = np.stack([
        np.stack([one, a, b], axis=-1),
        np.stack([-a, one, c], axis=-1),
        np.stack([-b, -c, one], axis=-1),
    ], axis=-2)
    G8 = _inv3(first) @ second
    eye = np.eye(3, dtype=k24.dtype)
    return np.stack([G8[0], G8[1], G8[2], G8[3], eye, G8[5], G8[6], G8[7], G8[4]], axis=0)


def _host_prep(g):
    """W [9,126,126]: rows (c'*9+jl), cols (im*14+c'); c2s [2,128,256]."""
    G = _cayley(g[:24].astype(np.float64))
    Mmat = np.einsum('kij,kml->kimjl', G, G).reshape(9, 9, 9)  # [k, im, jl]
    Wmat = np.zeros((9, PTF, PTF), dtype=np.float64)
    for cp in range(CPB):
        # rows cp*9 + jl, cols im*14 + cp
        Wmat[:, cp * 9:cp * 9 + 9, cp::CPB] = Mmat.transpose(0, 2, 1)  # [k, jl, im]
    c2 = np.square(g[24:].astype(np.float64)).reshape(C_IN, C_OUT)
    return Wmat.astype(np.float32), c2.reshape(2, 128, C_OUT).astype(np.float32)


# ---------------- device kernel ----------------
def _build_kernel(tc, xr, wk, c2, out):
    from contextlib import ExitStack
    ctx = ExitStack()
    nc = tc.nc
    f32 = mybir.dt.float32
    mdt = mybir.dt.float32r if USE_F32R else f32

    def mm(ap):
        return ap

    const = ctx.enter_context(tc.tile_pool(name="const", bufs=1))
    xpool = ctx.enter_context(tc.tile_pool(name="xpool", bufs=3))
    opool = ctx.enter_context(tc.tile_pool(name="opool", bufs=3))
    o2pool = ctx.enter_context(tc.tile_pool(name="o2pool", bufs=1))
    outp = ctx.enter_context(tc.tile_pool(name="outp", bufs=3))
    pspool = ctx.enter_context(tc.tile_pool(name="pspool", bufs=2, space="PSUM"))
    ps2pool = ctx.enter_context(tc.tile_pool(name="ps2pool", bufs=4, space="PSUM"))

    # constants
    wk_sb = const.tile([PTF, 9, PTF], mdt, name="wk_sb")
    nc.sync.dma_start(out=wk_sb[:], in_=wk.rearrange("k p m -> p k m"))
    c2_sb = const.tile([128, 2, C_OUT], mdt, name="c2_sb")
    nc.sync.dma_start(out=c2_sb[:], in_=c2.rearrange("t p o -> p t o"))

    # o2[c_chunk][c_part, im, pix2]
    o2a = o2pool.tile([128, 9, PIX2], mdt, name="o2a")
    o2b = o2pool.tile([128, 9, PIX2], mdt, name="o2b")

    # ---- transform + fold ----
    # xt is flat pitch-30: row r = h+3 (3 top pad rows, 2 bottom), cols 0..27
    # data + 2 zero pad cols. The fold shift (a,b) is then the flat shift
    # -(a*30+b); wrap columns land in the zero pads.
    for cb in range(NCB):
        xt = xpool.tile([PTF, 990], mdt, name="xt")
        nc.sync.dma_start(out=xt[:], in_=xr[cb])
        for half in range(2):
            ps = pspool.tile([PTF, 450], f32, name="ps")
            for k in range(9):
                a, b = divmod(k, 3)
                q0 = (15 * half + 3 - a) * 30 - b
                nc.tensor.matmul(ps[:], mm(wk_sb[:, k, :]), mm(xt[:, q0:q0 + 450]),
                                 start=(k == 0), stop=(k == 8))
            osb = opool.tile([PTF, 450], mdt, name="osb")
            nc.vector.tensor_copy(out=osb[:], in_=ps[:])
            # reorg partitions (im*14+c') -> o2[c = cb*14+c', im, half*450:+450]
            c_lo = cb * CPB
            nreal = CPB if cb < NCB - 1 else C_IN - c_lo
            pieces = []
            if c_lo < 128:
                hi = min(128, c_lo + nreal)
                pieces.append((o2a, c_lo, 0, hi - c_lo))
            if c_lo + nreal > 128:
                lo = max(128, c_lo)
                pieces.append((o2b, lo - 128, lo - c_lo, c_lo + nreal - lo))
            for (o2t, p0, cp0, ncp) in pieces:
                for im in range(9):
                    src = osb[im * CPB + cp0:im * CPB + cp0 + ncp, :]
                    dst = o2t[p0:p0 + ncp, im, 450 * half:450 * (half + 1)]
                    nc.sync.dma_start(out=dst, in_=src)

    # ---- channel mix ----
    # out rows = p2*9 + im; loop im, chunk p2 by 126 -> strided-row DMA out
    outr = out.rearrange("(p i) o -> p i o", i=9)
    for j in range((PIX2 + 125) // 126):
        p0 = j * 126
        npx = min(126, PIX2 - p0)
        for im in range(9):
            ps2 = ps2pool.tile([PTF, C_OUT], f32, name="ps2")
            for t, o2t in enumerate((o2a, o2b)):
                nc.tensor.matmul(ps2[:npx, :], mm(o2t[:, im, p0:p0 + npx]),
                                 mm(c2_sb[:, t, :]), start=(t == 0), stop=(t == 1))
            ob = outp.tile([PTF, C_OUT], f32, name="ob")
            nc.vector.tensor_copy(out=ob[:npx, :], in_=ps2[:npx, :])
            nc.sync.dma_start(out=outr[p0:p0 + npx, im, :], in_=ob[:npx, :])

    ctx.close()


_MDT = mybir.dt.float32r if USE_F32R else mybir.dt.float32

_NC_CACHE = None


def _get_nc():
    global _NC_CACHE
    if _NC_CACHE is None:
        nc = bacc.Bacc("TRN2", target_bir_lowering=False, debug=False, num_devices=8)
        xr = nc.dram_tensor("xr", [NCB, PTF, 990], _MDT, kind="ExternalInput").ap()
        wk = nc.dram_tensor("wk", [9, PTF, PTF], _MDT, kind="ExternalInput").ap()
        c2 = nc.dram_tensor("c2", [2, 128, C_OUT], _MDT, kind="ExternalInput").ap()
        out = nc.dram_tensor("out", [NROW, C_OUT], mybir.dt.float32, kind="ExternalOutput").ap()
        with tile.TileContext(nc) as tc:
            _build_kernel(tc, xr, wk, c2, out)
        nc.compile()
        _NC_CACHE = nc
    return _NC_CACHE


def _shard_inputs(x, g):
    x = np.ascontiguousarray(np.asarray(x, dtype=np.float32))
    g = np.asarray(g, dtype=np.float32)
    Wmat, c2s = _host_prep(g)
    # xr[b, cb, c'*9+jl, (3+h)*30+w] = x[b, cb*14+c', h, w, jl]; 33x30 zero-pad
    xp = np.zeros((B, NCB * CPB, 9, 33, 30), dtype=np.float32)
    xp[:, :C_IN, :, 3:31, 0:28] = x.transpose(0, 1, 4, 2, 3)
    xr = xp.reshape(B, NCB, PTF, 990)
    return [{"xr": np.ascontiguousarray(xr[b]), "wk": Wmat, "c2": c2s} for b in range(B)], Wmat


def kernel(x, g, _want_profile=False):
    nc = _get_nc()
    in_maps, _ = _shard_inputs(x, g)
    res = run_bass_kernel_spmd(nc, in_maps, list(range(B)), trace=_want_profile)
    outs = np.stack([res.results[b]["out"] for b in range(B)], axis=0)
    full = outs.reshape(B, 30, 30, 3, 3, C_OUT).astype(np.float32)
    if _want_profile:
        return full, res
    return full


# revision 10
# speedup vs baseline: 2.2644x; 2.2644x over previous
"""Trainium2 Bass kernel for nn_CayleyConv.

Reference computation (per batch b):
  G = cayley(g[:24])                                   # [9,3,3]
  Y[c,h,w,k] = G[k] @ X[c,h,w] @ G[k]^T                # 3x3 mats per pixel
  O[c,h2,w2] = sum_k Y[c,h2-a,w2-b,k]  (k=(a,b))       # 3x3 overlap-add fold
  out[h2,w2,:,:,o] = sum_c O[c,h2,w2] * c2[c,o]        # channel mix

Strategy (data-parallel over batch, 1 batch per NeuronCore):
  * transform+fold in ONE PE pass: stationary W_k[(c',jl),(im,c')] =
    delta_{c'} * M_k[im,jl] (M_k = kron(G_k,G_k)) gives K=126 contraction
    (14 channels x 9 matrix components on partitions), fold realized by
    9 shift-accumulated matmuls into one PSUM tile per 15-row output half.
  * input zero-padded to 32x32 spatially so every fold matmul writes the
    identical full PSUM region (uniform has_written semantics).
  * partition reorg (im,c') -> c via on-chip DMA into o2 [c, im, pix2].
  * channel mix: lhsT = o2 2D-chunk (pix2-major, im-minor = final row
    order), rhs = c2 [c,256]; psum-accumulate the two 128-channel halves.
  * float32r matmuls (full-rate fp32 streaming for N>=256).
"""

import numpy as np

import concourse.bass as bass
import concourse.mybir as mybir
import concourse.tile as tile
from concourse import bacc
from concourse.bass_utils import run_bass_kernel_spmd

# ---------------- problem constants (hardcoded per contract) ----------------
B = 8
C_IN = 256
C_OUT = 256
H = 28
W = 28
NCB = 19          # ceil(256/14) channel blocks
CPB = 14          # channels per block
PTF = 126         # CPB * 9 partitions for transform+fold
PIX2 = 900        # 30*30 output pixels
NROW = PIX2 * 9   # 8100 output rows (pix2-major, im-minor)
EPS = 1e-7

USE_F32R = True   # float32r matmuls (1 cyc/row at N>=256) vs float32 (4 cyc/row)


# ---------------- host-side param prep ----------------
def _inv3(m):
    b00, b01, b02 = m[..., 0, 0], m[..., 0, 1], m[..., 0, 2]
    b10, b11, b12 = m[..., 1, 0], m[..., 1, 1], m[..., 1, 2]
    b20, b21, b22 = m[..., 2, 0], m[..., 2, 1], m[..., 2, 2]
    det = (b00 * (b11 * b22 - b12 * b21)
           - b01 * (b10 * b22 - b12 * b20)
           + b02 * (b10 * b21 - b11 * b20))
    adj = np.stack([
        np.stack([b11 * b22 - b12 * b21, b02 * b21 - b01 * b22, b01 * b12 - b02 * b11], axis=-1),
        np.stack([b12 * b20 - b10 * b22, b00 * b22 - b02 * b20, b02 * b10 - b00 * b12], axis=-1),
        np.stack([b10 * b21 - b11 * b20, b01 * b20 - b00 * b21, b00 * b11 - b01 * b10], axis=-1),
    ], axis=-2)
    return adj / (det + EPS)[..., None, None]


def _cayley(k24):
    p = k24.reshape(8, 3)
    a, b, c = p[:, 0], p[:, 1], p[:, 2]
    one = np.ones_like(a)
    first = np.stack([
        np.stack([one, -a, -b], axis=-1),
        np.stack([a, one, -c], axis=-1),
        np.stack([b, c, one], axis=-1),
    ], axis=-2)
    second = np.stack([
        np.stack([one, a, b], axis=-1),
        np.stack([-a, one, c], axis=-1),
        np.stack([-b, -c, one], axis=-1),
    ], axis=-2)
    G8 = _inv3(first) @ second
    eye = np.eye(3, dtype=k24.dtype)
    return np.stack([G8[0], G8[1], G8[2], G8[3], eye, G8[5], G8[6], G8[7], G8[4]], axis=0)


def _host_prep(g):
    """W [9,126,126]: rows (c'*9+jl), cols (im*14+c'); c2s [2,128,256]."""
    G = _cayley(g[:24].astype(np.float64))
    Mmat = np.einsum('kij,kml->kimjl', G, G).reshape(9, 9, 9)  # [k, im, jl]
    Wmat = np.zeros((9, PTF, PTF), dtype=np.float64)
    for cp in range(CPB):
        # rows cp*9 + jl, cols im*14 + cp
        Wmat[:, cp * 9:cp * 9 + 9, cp::CPB] = Mmat.transpose(0, 2, 1)  # [k, jl, im]
    c2 = np.square(g[24:].astype(np.float64)).reshape(C_IN, C_OUT)
    return Wmat.astype(np.float32), c2.reshape(2, 128, C_OUT).astype(np.float32)


# ---------------- device kernel ----------------
def _build_kernel(tc, xr, wk, c2, out):
    from contextlib import ExitStack
    ctx = ExitStack()
    nc = tc.nc
    f32 = mybir.dt.float32
    mdt = mybir.dt.float32r if USE_F32R else f32

    const = ctx.enter_context(tc.tile_pool(name="const", bufs=1))
    xpool = ctx.enter_context(tc.tile_pool(name="xpool", bufs=3))
    opool = ctx.enter_context(tc.tile_pool(name="opool", bufs=3))
    o2pool = ctx.enter_context(tc.tile_pool(name="o2pool", bufs=1))
    outp = ctx.enter_context(tc.tile_pool(name="outp", bufs=3))
    pspool = ctx.enter_context(tc.tile_pool(name="pspool", bufs=2, space="PSUM"))
    ps2pool = ctx.enter_context(tc.tile_pool(name="ps2pool", bufs=4, space="PSUM"))
    dram = ctx.enter_context(tc.tile_pool(name="dram", bufs=1, space="DRAM"))

    # constants
    wk_sb = const.tile([PTF, 9, PTF], mdt, name="wk_sb")
    nc.sync.dma_start(out=wk_sb[:], in_=wk.rearrange("k p m -> p k m"))
    c2_sb = const.tile([128, 2, C_OUT], mdt, name="c2_sb")
    nc.sync.dma_start(out=c2_sb[:], in_=c2.rearrange("t p o -> p t o"))

    # o2[c_chunk][c_part=c%128, im, pix2] for c chunks 0-127 / 128-255
    o2a = o2pool.tile([128, 9, PIX2], mdt, name="o2a")
    o2b = o2pool.tile([128, 9, PIX2], mdt, name="o2b")
    # DRAM bounce for the (im,c')->c partition reorg
    obounce = dram.tile([NCB, PTF, PIX2], mdt, name="obounce")

    # ---- transform + fold ----
    # xt is flat pitch-30: row r = h+3 (3 top pad rows, 2 bottom), cols 0..27
    # data + 2 zero pad cols. The fold shift (a,b) is then the flat shift
    # -(a*30+b); wrap columns land in the zero pads.
    for cb in range(NCB):
        xt = xpool.tile([PTF, 990], mdt, name="xt")
        nc.sync.dma_start(out=xt[:], in_=xr[cb])
        osb = opool.tile([PTF, PIX2], mdt, name="osb")
        for half in range(2):
            ps = pspool.tile([PTF, 450], f32, name="ps")
            for k in range(9):
                a, b = divmod(k, 3)
                q0 = (15 * half + 3 - a) * 30 - b
                nc.tensor.matmul(ps[:], wk_sb[:, k, :], xt[:, q0:q0 + 450],
                                 start=(k == 0), stop=(k == 8))
            nc.vector.tensor_copy(out=osb[:, 450 * half:450 * (half + 1)], in_=ps[:])
        nc.scalar.dma_start(out=obounce[cb], in_=osb[:])

    # ---- gather loads: o2[c, im, :] = obounce[cb(c), im*14+c'(c), :] ----
    ob9 = obounce[:].rearrange("cb (im cp) t -> cb cp im t", cp=CPB)
    for cb in range(NCB):
        c_lo = cb * CPB
        nreal = CPB if cb < NCB - 1 else C_IN - c_lo
        pieces = []
        if c_lo < 128:
            hi = min(128, c_lo + nreal)
            pieces.append((o2a, c_lo, 0, hi - c_lo))
        if c_lo + nreal > 128:
            lo = max(128, c_lo)
            pieces.append((o2b, lo - 128, lo - c_lo, c_lo + nreal - lo))
        for (o2t, p0, cp0, ncp) in pieces:
            nc.scalar.dma_start(out=o2t[p0:p0 + ncp], in_=ob9[cb, cp0:cp0 + ncp])

    # ---- channel mix ----
    # out rows = p2*9 + im; chunk p2 by 126, all im staged then one DMA
    outr = out.rearrange("(p i) o -> p i o", i=9)
    for j in range((PIX2 + 125) // 126):
        p0 = j * 126
        npx = min(126, PIX2 - p0)
        ob3 = outp.tile([PTF, 9, C_OUT], f32, name="ob3")
        for im in range(9):
            ps2 = ps2pool.tile([PTF, C_OUT], f32, name="ps2")
            for t, o2t in enumerate((o2a, o2b)):
                nc.tensor.matmul(ps2[:npx, :], o2t[:, im, p0:p0 + npx],
                                 c2_sb[:, t, :], start=(t == 0), stop=(t == 1))
            nc.vector.tensor_copy(out=ob3[:npx, im, :], in_=ps2[:npx, :])
        nc.sync.dma_start(out=outr[p0:p0 + npx], in_=ob3[:npx])

    ctx.close()


_MDT = mybir.dt.float32r if USE_F32R else mybir.dt.float32

_NC_CACHE = None


def _get_nc():
    global _NC_CACHE
    if _NC_CACHE is None:
        nc = bacc.Bacc("TRN2", target_bir_lowering=False, debug=False, num_devices=8)
        xr = nc.dram_tensor("xr", [NCB, PTF, 990], _MDT, kind="ExternalInput").ap()
        wk = nc.dram_tensor("wk", [9, PTF, PTF], _MDT, kind="ExternalInput").ap()
        c2 = nc.dram_tensor("c2", [2, 128, C_OUT], _MDT, kind="ExternalInput").ap()
        out = nc.dram_tensor("out", [NROW, C_OUT], mybir.dt.float32, kind="ExternalOutput").ap()
        with tile.TileContext(nc) as tc:
            _build_kernel(tc, xr, wk, c2, out)
        nc.compile()
        _NC_CACHE = nc
    return _NC_CACHE


def _shard_inputs(x, g):
    x = np.ascontiguousarray(np.asarray(x, dtype=np.float32))
    g = np.asarray(g, dtype=np.float32)
    Wmat, c2s = _host_prep(g)
    # xr[b, cb, c'*9+jl, (3+h)*30+w] = x[b, cb*14+c', h, w, jl]; 33x30 zero-pad
    xp = np.zeros((B, NCB * CPB, 9, 33, 30), dtype=np.float32)
    xp[:, :C_IN, :, 3:31, 0:28] = x.transpose(0, 1, 4, 2, 3)
    xr = xp.reshape(B, NCB, PTF, 990)
    return [{"xr": np.ascontiguousarray(xr[b]), "wk": Wmat, "c2": c2s} for b in range(B)], Wmat


def kernel(x, g, _want_profile=False):
    nc = _get_nc()
    in_maps, _ = _shard_inputs(x, g)
    res = run_bass_kernel_spmd(nc, in_maps, list(range(B)), trace=_want_profile)
    outs = np.stack([res.results[b]["out"] for b in range(B)], axis=0)
    full = outs.reshape(B, 30, 30, 3, 3, C_OUT).astype(np.float32)
    if _want_profile:
        return full, res
    return full


# revision 11
# speedup vs baseline: 2.3760x; 1.0493x over previous
"""Trainium2 Bass kernel for nn_CayleyConv.

Reference computation (per batch b):
  G = cayley(g[:24])                                   # [9,3,3]
  Y[c,h,w,k] = G[k] @ X[c,h,w] @ G[k]^T                # 3x3 mats per pixel
  O[c,h2,w2] = sum_k Y[c,h2-a,w2-b,k]  (k=(a,b))       # 3x3 overlap-add fold
  out[h2,w2,:,:,o] = sum_c O[c,h2,w2] * c2[c,o]        # channel mix

Strategy (data-parallel over batch, 1 batch per NeuronCore):
  * transform+fold in ONE PE pass: stationary W_k[(c',jl),(im,c')] =
    delta_{c'} * M_k[im,jl] (M_k = kron(G_k,G_k)) gives K=126 contraction
    (14 channels x 9 matrix components on partitions), fold realized by
    9 shift-accumulated matmuls into one PSUM tile per 15-row output half.
  * input zero-padded to 32x32 spatially so every fold matmul writes the
    identical full PSUM region (uniform has_written semantics).
  * partition reorg (im,c') -> c via on-chip DMA into o2 [c, im, pix2].
  * channel mix: lhsT = o2 2D-chunk (pix2-major, im-minor = final row
    order), rhs = c2 [c,256]; psum-accumulate the two 128-channel halves.
  * float32r matmuls (full-rate fp32 streaming for N>=256).
"""

import numpy as np

import concourse.bass as bass
import concourse.mybir as mybir
import concourse.tile as tile
from concourse import bacc
from concourse.bass_utils import run_bass_kernel_spmd

# ---------------- problem constants (hardcoded per contract) ----------------
B = 8
C_IN = 256
C_OUT = 256
H = 28
W = 28
NCB = 19          # ceil(256/14) channel blocks
CPB = 14          # channels per block
PTF = 126         # CPB * 9 partitions for transform+fold
PIX2 = 900        # 30*30 output pixels
NROW = PIX2 * 9   # 8100 output rows (pix2-major, im-minor)
EPS = 1e-7

USE_F32R = True   # float32r matmuls (1 cyc/row at N>=256) vs float32 (4 cyc/row)


# ---------------- host-side param prep ----------------
def _inv3(m):
    b00, b01, b02 = m[..., 0, 0], m[..., 0, 1], m[..., 0, 2]
    b10, b11, b12 = m[..., 1, 0], m[..., 1, 1], m[..., 1, 2]
    b20, b21, b22 = m[..., 2, 0], m[..., 2, 1], m[..., 2, 2]
    det = (b00 * (b11 * b22 - b12 * b21)
           - b01 * (b10 * b22 - b12 * b20)
           + b02 * (b10 * b21 - b11 * b20))
    adj = np.stack([
        np.stack([b11 * b22 - b12 * b21, b02 * b21 - b01 * b22, b01 * b12 - b02 * b11], axis=-1),
        np.stack([b12 * b20 - b10 * b22, b00 * b22 - b02 * b20, b02 * b10 - b00 * b12], axis=-1),
        np.stack([b10 * b21 - b11 * b20, b01 * b20 - b00 * b21, b00 * b11 - b01 * b10], axis=-1),
    ], axis=-2)
    return adj / (det + EPS)[..., None, None]


def _cayley(k24):
    p = k24.reshape(8, 3)
    a, b, c = p[:, 0], p[:, 1], p[:, 2]
    one = np.ones_like(a)
    first = np.stack([
        np.stack([one, -a, -b], axis=-1),
        np.stack([a, one, -c], axis=-1),
        np.stack([b, c, one], axis=-1),
    ], axis=-2)
    second = np.stack([
        np.stack([one, a, b], axis=-1),
        np.stack([-a, one, c], axis=-1),
        np.stack([-b, -c, one], axis=-1),
    ], axis=-2)
    G8 = _inv3(first) @ second
    eye = np.eye(3, dtype=k24.dtype)
    return np.stack([G8[0], G8[1], G8[2], G8[3], eye, G8[5], G8[6], G8[7], G8[4]], axis=0)


def _host_prep(g):
    """W [9,126,126]: rows (c'*9+jl), cols (im*14+c'); c2s [2,128,256]."""
    G = _cayley(g[:24].astype(np.float64))
    Mmat = np.einsum('kij,kml->kimjl', G, G).reshape(9, 9, 9)  # [k, im, jl]
    Wmat = np.zeros((9, PTF, PTF), dtype=np.float64)
    for cp in range(CPB):
        # rows cp*9 + jl, cols im*14 + cp
        Wmat[:, cp * 9:cp * 9 + 9, cp::CPB] = Mmat.transpose(0, 2, 1)  # [k, jl, im]
    c2 = np.square(g[24:].astype(np.float64)).reshape(C_IN, C_OUT)
    return Wmat.astype(np.float32), c2.reshape(2, 128, C_OUT).astype(np.float32)


# ---------------- device kernel ----------------
def _build_kernel(tc, xr, wk, c2, out):
    from contextlib import ExitStack
    ctx = ExitStack()
    nc = tc.nc
    f32 = mybir.dt.float32
    mdt = mybir.dt.float32r if USE_F32R else f32

    const = ctx.enter_context(tc.tile_pool(name="const", bufs=1))
    xpool = ctx.enter_context(tc.tile_pool(name="xpool", bufs=4))
    opool = ctx.enter_context(tc.tile_pool(name="opool", bufs=3))
    o2pool = ctx.enter_context(tc.tile_pool(name="o2pool", bufs=1))
    outp = ctx.enter_context(tc.tile_pool(name="outp", bufs=3))
    pspool = ctx.enter_context(tc.tile_pool(name="pspool", bufs=3, space="PSUM"))
    ps2pool = ctx.enter_context(tc.tile_pool(name="ps2pool", bufs=4, space="PSUM"))
    dram = ctx.enter_context(tc.tile_pool(name="dram", bufs=1, space="DRAM"))

    # constants
    wk_sb = const.tile([PTF, 9, PTF], mdt, name="wk_sb")
    nc.sync.dma_start(out=wk_sb[:], in_=wk.rearrange("k p m -> p k m"))
    c2_sb = const.tile([128, 2, C_OUT], mdt, name="c2_sb")
    nc.sync.dma_start(out=c2_sb[:], in_=c2.rearrange("t p o -> p t o"))

    # o2[c_chunk][c_part=c%128, im, pix2] for c chunks 0-127 / 128-255
    o2a = o2pool.tile([128, 9, PIX2], mdt, name="o2a")
    o2b = o2pool.tile([128, 9, PIX2], mdt, name="o2b")
    # DRAM bounce for the (im,c')->c partition reorg
    obounce = dram.tile([NCB, PTF, PIX2], mdt, name="obounce")

    # ---- transform + fold ----
    # xt is flat pitch-30: row r = h+3 (3 top pad rows, 2 bottom), cols 0..27
    # data + 2 zero pad cols. The fold shift (a,b) is then the flat shift
    # -(a*30+b); wrap columns land in the zero pads.
    for cb in range(NCB):
        xt = xpool.tile([PTF, 990], mdt, name="xt")
        nc.sync.dma_start(out=xt[:], in_=xr[cb])
        osb = opool.tile([PTF, PIX2], mdt, name="osb")
        for half in range(2):
            ps = pspool.tile([PTF, 450], f32, name="ps")
            for k in range(9):
                a, b = divmod(k, 3)
                q0 = (15 * half + 3 - a) * 30 - b
                nc.tensor.matmul(ps[:], wk_sb[:, k, :], xt[:, q0:q0 + 450],
                                 start=(k == 0), stop=(k == 8))
            nc.vector.tensor_copy(out=osb[:, 450 * half:450 * (half + 1)], in_=ps[:])
        nc.scalar.dma_start(out=obounce[cb], in_=osb[:])
        # gather immediately: o2[c, im, :] = obounce[cb, im*14+c', :]
        ob9 = obounce[:].rearrange("cb (im cp) t -> cb cp im t", cp=CPB)
        c_lo = cb * CPB
        nreal = CPB if cb < NCB - 1 else C_IN - c_lo
        pieces = []
        if c_lo < 128:
            hi = min(128, c_lo + nreal)
            pieces.append((o2a, c_lo, 0, hi - c_lo))
        if c_lo + nreal > 128:
            lo = max(128, c_lo)
            pieces.append((o2b, lo - 128, lo - c_lo, c_lo + nreal - lo))
        for (o2t, p0, cp0, ncp) in pieces:
            nc.scalar.dma_start(out=o2t[p0:p0 + ncp], in_=ob9[cb, cp0:cp0 + ncp])

    # ---- channel mix ----
    # out rows = p2*9 + im; chunk p2 by 126, all im staged then one DMA
    outr = out.rearrange("(p i) o -> p i o", i=9)
    for j in range((PIX2 + 125) // 126):
        p0 = j * 126
        npx = min(126, PIX2 - p0)
        ob3 = outp.tile([PTF, 9, C_OUT], f32, name="ob3")
        for im in range(9):
            ps2 = ps2pool.tile([PTF, C_OUT], f32, name="ps2")
            for t, o2t in enumerate((o2a, o2b)):
                nc.tensor.matmul(ps2[:npx, :], o2t[:, im, p0:p0 + npx],
                                 c2_sb[:, t, :], start=(t == 0), stop=(t == 1))
            nc.vector.tensor_copy(out=ob3[:npx, im, :], in_=ps2[:npx, :])
        nc.sync.dma_start(out=outr[p0:p0 + npx], in_=ob3[:npx])

    ctx.close()


_MDT = mybir.dt.float32r if USE_F32R else mybir.dt.float32

_NC_CACHE = None


def _get_nc():
    global _NC_CACHE
    if _NC_CACHE is None:
        nc = bacc.Bacc("TRN2", target_bir_lowering=False, debug=False, num_devices=8)
        xr = nc.dram_tensor("xr", [NCB, PTF, 990], _MDT, kind="ExternalInput").ap()
        wk = nc.dram_tensor("wk", [9, PTF, PTF], _MDT, kind="ExternalInput").ap()
        c2 = nc.dram_tensor("c2", [2, 128, C_OUT], _MDT, kind="ExternalInput").ap()
        out = nc.dram_tensor("out", [NROW, C_OUT], mybir.dt.float32, kind="ExternalOutput").ap()
        with tile.TileContext(nc) as tc:
            _build_kernel(tc, xr, wk, c2, out)
        nc.compile()
        _NC_CACHE = nc
    return _NC_CACHE


def _shard_inputs(x, g):
    x = np.ascontiguousarray(np.asarray(x, dtype=np.float32))
    g = np.asarray(g, dtype=np.float32)
    Wmat, c2s = _host_prep(g)
    # xr[b, cb, c'*9+jl, (3+h)*30+w] = x[b, cb*14+c', h, w, jl]; 33x30 zero-pad
    xp = np.zeros((B, NCB * CPB, 9, 33, 30), dtype=np.float32)
    xp[:, :C_IN, :, 3:31, 0:28] = x.transpose(0, 1, 4, 2, 3)
    xr = xp.reshape(B, NCB, PTF, 990)
    return [{"xr": np.ascontiguousarray(xr[b]), "wk": Wmat, "c2": c2s} for b in range(B)], Wmat


def kernel(x, g, _want_profile=False):
    nc = _get_nc()
    in_maps, _ = _shard_inputs(x, g)
    res = run_bass_kernel_spmd(nc, in_maps, list(range(B)), trace=_want_profile)
    outs = np.stack([res.results[b]["out"] for b in range(B)], axis=0)
    full = outs.reshape(B, 30, 30, 3, 3, C_OUT).astype(np.float32)
    if _want_profile:
        return full, res
    return full


# revision 12
# speedup vs baseline: 2.6817x; 1.1287x over previous
"""Trainium2 Bass kernel for nn_CayleyConv.

Reference computation (per batch b):
  G = cayley(g[:24])                                   # [9,3,3]
  Y[c,h,w,k] = G[k] @ X[c,h,w] @ G[k]^T                # 3x3 mats per pixel
  O[c,h2,w2] = sum_k Y[c,h2-a,w2-b,k]  (k=(a,b))       # 3x3 overlap-add fold
  out[h2,w2,:,:,o] = sum_c O[c,h2,w2] * c2[c,o]        # channel mix

Strategy (data-parallel over batch, 1 batch per NeuronCore):
  * transform+fold in ONE PE pass: stationary W_k[(c',jl),(im,c')] =
    delta_{c'} * M_k[im,jl] (M_k = kron(G_k,G_k)) gives K=126 contraction
    (14 channels x 9 matrix components on partitions), fold realized by
    9 shift-accumulated matmuls into one PSUM tile per 15-row output half.
  * input zero-padded to 32x32 spatially so every fold matmul writes the
    identical full PSUM region (uniform has_written semantics).
  * partition reorg (im,c') -> c via on-chip DMA into o2 [c, im, pix2].
  * channel mix: lhsT = o2 2D-chunk (pix2-major, im-minor = final row
    order), rhs = c2 [c,256]; psum-accumulate the two 128-channel halves.
  * float32r matmuls (full-rate fp32 streaming for N>=256).
"""

import numpy as np

import concourse.bass as bass
import concourse.mybir as mybir
import concourse.tile as tile
from concourse import bacc
from concourse.bass_utils import run_bass_kernel_spmd

# ---------------- problem constants (hardcoded per contract) ----------------
B = 8
C_IN = 256
C_OUT = 256
H = 28
W = 28
NCB = 19          # ceil(256/14) channel blocks
CPB = 14          # channels per block
PTF = 126         # CPB * 9 partitions for transform+fold
PIX2 = 900        # 30*30 output pixels
NROW = PIX2 * 9   # 8100 output rows (pix2-major, im-minor)
EPS = 1e-7

USE_F32R = True   # float32r matmuls (1 cyc/row at N>=256) vs float32 (4 cyc/row)


# ---------------- host-side param prep ----------------
def _inv3(m):
    b00, b01, b02 = m[..., 0, 0], m[..., 0, 1], m[..., 0, 2]
    b10, b11, b12 = m[..., 1, 0], m[..., 1, 1], m[..., 1, 2]
    b20, b21, b22 = m[..., 2, 0], m[..., 2, 1], m[..., 2, 2]
    det = (b00 * (b11 * b22 - b12 * b21)
           - b01 * (b10 * b22 - b12 * b20)
           + b02 * (b10 * b21 - b11 * b20))
    adj = np.stack([
        np.stack([b11 * b22 - b12 * b21, b02 * b21 - b01 * b22, b01 * b12 - b02 * b11], axis=-1),
        np.stack([b12 * b20 - b10 * b22, b00 * b22 - b02 * b20, b02 * b10 - b00 * b12], axis=-1),
        np.stack([b10 * b21 - b11 * b20, b01 * b20 - b00 * b21, b00 * b11 - b01 * b10], axis=-1),
    ], axis=-2)
    return adj / (det + EPS)[..., None, None]


def _cayley(k24):
    p = k24.reshape(8, 3)
    a, b, c = p[:, 0], p[:, 1], p[:, 2]
    one = np.ones_like(a)
    first = np.stack([
        np.stack([one, -a, -b], axis=-1),
        np.stack([a, one, -c], axis=-1),
        np.stack([b, c, one], axis=-1),
    ], axis=-2)
    second = np.stack([
        np.stack([one, a, b], axis=-1),
        np.stack([-a, one, c], axis=-1),
        np.stack([-b, -c, one], axis=-1),
    ], axis=-2)
    G8 = _inv3(first) @ second
    eye = np.eye(3, dtype=k24.dtype)
    return np.stack([G8[0], G8[1], G8[2], G8[3], eye, G8[5], G8[6], G8[7], G8[4]], axis=0)


def _host_prep(g):
    """W [9,126,126]: rows (c'*9+jl), cols (im*14+c'); c2s [2,128,256]."""
    G = _cayley(g[:24].astype(np.float64))
    Mmat = np.einsum('kij,kml->kimjl', G, G).reshape(9, 9, 9)  # [k, im, jl]
    Wmat = np.zeros((9, PTF, PTF), dtype=np.float64)
    for cp in range(CPB):
        # rows cp*9 + jl, cols im*14 + cp
        Wmat[:, cp * 9:cp * 9 + 9, cp::CPB] = Mmat.transpose(0, 2, 1)  # [k, jl, im]
    c2 = np.square(g[24:].astype(np.float64)).reshape(C_IN, C_OUT)
    return Wmat.astype(np.float32), c2.reshape(2, 128, C_OUT).astype(np.float32)


# ---------------- device kernel ----------------
def _build_kernel(tc, xr, wk, c2, out):
    from contextlib import ExitStack
    ctx = ExitStack()
    nc = tc.nc
    f32 = mybir.dt.float32
    mdt = mybir.dt.float32r if USE_F32R else f32

    const = ctx.enter_context(tc.tile_pool(name="const", bufs=1))
    xpool = ctx.enter_context(tc.tile_pool(name="xpool", bufs=4))
    opool = ctx.enter_context(tc.tile_pool(name="opool", bufs=3))
    o2pool = ctx.enter_context(tc.tile_pool(name="o2pool", bufs=1))
    outp = ctx.enter_context(tc.tile_pool(name="outp", bufs=3))
    pspool = ctx.enter_context(tc.tile_pool(name="pspool", bufs=3, space="PSUM"))
    ps2pool = ctx.enter_context(tc.tile_pool(name="ps2pool", bufs=4, space="PSUM"))
    dram = ctx.enter_context(tc.tile_pool(name="dram", bufs=1, space="DRAM"))

    # constants
    wk_sb = const.tile([PTF, 9, PTF], mdt, name="wk_sb")
    nc.sync.dma_start(out=wk_sb[:], in_=wk.rearrange("k p m -> p k m"))
    c2_sb = const.tile([128, 2, C_OUT], mdt, name="c2_sb")
    nc.sync.dma_start(out=c2_sb[:], in_=c2.rearrange("t p o -> p t o"))

    # o2[c_chunk][c_part=c%128, im, pix2] for c chunks 0-127 / 128-255
    o2a = o2pool.tile([128, 9, PIX2], mdt, name="o2a")
    o2b = o2pool.tile([128, 9, PIX2], mdt, name="o2b")
    # DRAM bounce for the (im,c')->c partition reorg
    obounce = dram.tile([NCB, PTF, PIX2], mdt, name="obounce")

    # ---- transform + fold ----
    # xt is flat pitch-30: row r = h+3 (3 top pad rows, 2 bottom), cols 0..27
    # data + 2 zero pad cols. The fold shift (a,b) is then the flat shift
    # -(a*30+b); wrap columns land in the zero pads.
    for cb in range(NCB):
        xt = xpool.tile([PTF, 990], mdt, name="xt")
        nc.sync.dma_start(out=xt[:], in_=xr[cb])
        osb = opool.tile([PTF, PIX2], mdt, name="osb")
        for half in range(2):
            ps = pspool.tile([PTF, 450], f32, name="ps")
            for k in range(9):
                a, b = divmod(k, 3)
                q0 = (15 * half + 3 - a) * 30 - b
                nc.tensor.matmul(ps[:], wk_sb[:, k, :], xt[:, q0:q0 + 450],
                                 start=(k == 0), stop=(k == 8))
            nc.vector.tensor_copy(out=osb[:, 450 * half:450 * (half + 1)], in_=ps[:])
        nc.scalar.dma_start(out=obounce[cb], in_=osb[:])
        # gather immediately: o2[c, im, :] = obounce[cb, im*14+c', :]
        ob9 = obounce[:].rearrange("cb (im cp) t -> cb cp im t", cp=CPB)
        c_lo = cb * CPB
        nreal = CPB if cb < NCB - 1 else C_IN - c_lo
        pieces = []
        if c_lo < 128:
            hi = min(128, c_lo + nreal)
            pieces.append((o2a, c_lo, 0, hi - c_lo))
        if c_lo + nreal > 128:
            lo = max(128, c_lo)
            pieces.append((o2b, lo - 128, lo - c_lo, c_lo + nreal - lo))
        for (o2t, p0, cp0, ncp) in pieces:
            nc.gpsimd.dma_start(out=o2t[p0:p0 + ncp], in_=ob9[cb, cp0:cp0 + ncp])

    # ---- channel mix ----
    # out rows = p2*9 + im; chunk p2 by 126, all im staged then one DMA
    outr = out.rearrange("(p i) o -> p i o", i=9)
    for j in range((PIX2 + 125) // 126):
        p0 = j * 126
        npx = min(126, PIX2 - p0)
        ob3 = outp.tile([PTF, 9, C_OUT], f32, name="ob3")
        for im in range(9):
            ps2 = ps2pool.tile([PTF, C_OUT], f32, name="ps2")
            for t, o2t in enumerate((o2a, o2b)):
                nc.tensor.matmul(ps2[:npx, :], o2t[:, im, p0:p0 + npx],
                                 c2_sb[:, t, :], start=(t == 0), stop=(t == 1))
            nc.vector.tensor_copy(out=ob3[:npx, im, :], in_=ps2[:npx, :])
        nc.sync.dma_start(out=outr[p0:p0 + npx], in_=ob3[:npx])

    ctx.close()


_MDT = mybir.dt.float32r if USE_F32R else mybir.dt.float32

_NC_CACHE = None


def _get_nc():
    global _NC_CACHE
    if _NC_CACHE is None:
        nc = bacc.Bacc("TRN2", target_bir_lowering=False, debug=False, num_devices=8)
        xr = nc.dram_tensor("xr", [NCB, PTF, 990], _MDT, kind="ExternalInput").ap()
        wk = nc.dram_tensor("wk", [9, PTF, PTF], _MDT, kind="ExternalInput").ap()
        c2 = nc.dram_tensor("c2", [2, 128, C_OUT], _MDT, kind="ExternalInput").ap()
        out = nc.dram_tensor("out", [NROW, C_OUT], mybir.dt.float32, kind="ExternalOutput").ap()
        with tile.TileContext(nc) as tc:
            _build_kernel(tc, xr, wk, c2, out)
        nc.compile()
        _NC_CACHE = nc
    return _NC_CACHE


def _shard_inputs(x, g):
    x = np.ascontiguousarray(np.asarray(x, dtype=np.float32))
    g = np.asarray(g, dtype=np.float32)
    Wmat, c2s = _host_prep(g)
    # xr[b, cb, c'*9+jl, (3+h)*30+w] = x[b, cb*14+c', h, w, jl]; 33x30 zero-pad
    xp = np.zeros((B, NCB * CPB, 9, 33, 30), dtype=np.float32)
    xp[:, :C_IN, :, 3:31, 0:28] = x.transpose(0, 1, 4, 2, 3)
    xr = xp.reshape(B, NCB, PTF, 990)
    return [{"xr": np.ascontiguousarray(xr[b]), "wk": Wmat, "c2": c2s} for b in range(B)], Wmat


def kernel(x, g, _want_profile=False):
    nc = _get_nc()
    in_maps, _ = _shard_inputs(x, g)
    res = run_bass_kernel_spmd(nc, in_maps, list(range(B)), trace=_want_profile)
    outs = np.stack([res.results[b]["out"] for b in range(B)], axis=0)
    full = outs.reshape(B, 30, 30, 3, 3, C_OUT).astype(np.float32)
    if _want_profile:
        return full, res
    return full
